# revision 26
# baseline (speedup 1.0000x reference)
"""Trainium2 Bass kernel for nn_Encoder_55688545960036.

Network: pointnet-style shared MLP (3->64->128, eval-mode BN folded into the
weights on the host, relu), 16 branch matmuls 128->1024 with folded BN and a
maxpool over the 2048 points of each batch element, squash over the branch
axis, capsule transform u[b,o,i,v] = sum_e caps[b,i,e] * Wc[o,i,e,v], 3 rounds
of dynamic routing, squash -> [4,32,32] output.

Distribution over 8 NeuronCores:
  phase A: branch axis k=16 -> 2 branches per core (shared MLP replicated).
  phase B: in-capsule axis i=1024 -> 128 per core (Wc 67MB -> 8.4MB/core).
  collectives: one AllToAll that converts per-core (2 branches, all 1024 i)
  feat into per-core (all 16 branches, 128-i shard), then 3 AllReduces of the
  routing partial sums s[4,32,32] (one per routing iteration).

All heavy matmuls run in float32r (full PE rate, ~1.5e-4 rel err on HW).
The maxpool is fused into PSUM evacuation with tensor_scalar accum max.
The capsule einsum is done as 16 block-diagonal matmuls (8 capsules of the
i-shard per group, caps entries on the block diagonal) with PE column tiling.
Routing keeps (i,b) on partitions and (o,v) in the free dims so the softmax
over o is a free-axis op; sums over i (partitions) are tiny selector matmuls.
"""

import functools

import numpy as np
from contextlib import ExitStack

import concourse.bass as bass
import concourse.tile as tile
from concourse import bacc, mybir
from concourse import bass_utils

# All ACT functions this kernel uses (Relu/Copy/Square/Ln/Exp/Identity) live
# together in the "natural_log_exp_and_others" table set, but the act-table
# placement pass greedily picks the FIRST set containing each func, which
# ping-pongs between Ln-only and Exp-only sets (one 1.3us table load per
# switch). Empty every other set (preserving dict order so walrus's
# act_func_set_id mapping is unchanged) so all activations resolve to the one
# set and a single load suffices.
_KEEP_ACT_SET = "natural_log_exp_and_others"
_orig_gat = bacc.get_activation_tables


@functools.cache
def _patched_gat(arch):
    t = _orig_gat(arch)
    return {k: (v if k == _KEEP_ACT_SET else set()) for k, v in t.items()}


bacc.get_activation_tables = _patched_gat

EPS = 1e-5
N_CORES = 8
B = 4
BN_ = 4 * 2048  # 8192 points
f32 = mybir.dt.float32
f32r = mybir.dt.float32r
AL = mybir.AluOpType
AF = mybir.ActivationFunctionType
AX = mybir.AxisListType
bf16 = mybir.dt.bfloat16
_BF = mybir.dt.np(bf16)

_CACHE = {}

# maxpool evacuation paths: D = DVE-direct-from-psum, A = ACT-copy +
# DVE remax (bf16, 4x), P = Pool(GpSimd)-direct-from-psum. Weighted
# round-robin so DVE/ACT/Pool busy times balance against the PE.
_PATH_W = {"A": 7, "D": 4, "P": 5}


def _make_paths(n=128):
    paths, cnt = [], {k: 0 for k in _PATH_W}
    for i in range(n):
        pick = max(_PATH_W, key=lambda k: _PATH_W[k] * (i + 1) / 16 - cnt[k])
        paths.append(pick)
        cnt[pick] += 1
    return paths


MAXPOOL_PATHS = _make_paths()


def _build_bass(reps=1, debug=False, stage=4, nocoll=False):
    # stage: 1=MLP only, 2=+branch/maxpool, 3=+A2A/caps/u, 4=full (routing)
    # nocoll: replace collectives with local DRAM copies (for TimelineSim)
    # reps > 1 replicates the compute body end-to-end inside one NEFF; used
    # only for wall-clock-difference timing in the dev harness.
    nc = bacc.Bacc("TRN2", target_bir_lowering=False, debug=False,
                   num_devices=N_CORES)

    # ---- DRAM I/O ----
    d_xT = nc.dram_tensor("xT", [3, BN_], f32r, kind="ExternalInput").ap()
    d_w1f = nc.dram_tensor("w1f", [3, 64], f32r, kind="ExternalInput").ap()
    d_c1f = nc.dram_tensor("c1f", [64, 1], f32, kind="ExternalInput").ap()
    d_w2f = nc.dram_tensor("w2f", [64, 128], f32r, kind="ExternalInput").ap()
    d_c2f = nc.dram_tensor("c2f", [128, 1], f32, kind="ExternalInput").ap()
    d_wbT = nc.dram_tensor("wbT", [128, 2048], f32r, kind="ExternalInput").ap()
    d_cb = nc.dram_tensor("cb", [128, 16], f32, kind="ExternalInput").ap()
    d_wc = nc.dram_tensor("wc", [16, 128, 1024], bf16, kind="ExternalInput").ap()
    d_sel132 = nc.dram_tensor("sel132", [128, 4], bf16, kind="ExternalInput").ap()
    d_sel1 = nc.dram_tensor("sel1", [128, 4], bf16, kind="ExternalInput").ap()
    d_sel4to128 = nc.dram_tensor("sel4to128", [4, 128], f32r,
                                 kind="ExternalInput").ap()
    d_selsq = nc.dram_tensor("selsq", [64, 4], f32r, kind="ExternalInput").ap()
    d_sel4to64 = nc.dram_tensor("sel4to64", [4, 64], f32r,
                                kind="ExternalInput").ap()
    d_espread = nc.dram_tensor("espread", [64, 512], bf16,
                               kind="ExternalInput").ap()
    d_maskd = nc.dram_tensor("maskd", [128, 512], bf16,
                             kind="ExternalInput").ap()
    d_ident = nc.dram_tensor("ident128", [128, 128], f32,
                             kind="ExternalInput").ap()
    # "out" holds this core's iteration-2 routing partial s2[b, (o,v)];
    # the host sums across cores and applies the final squash.
    d_out = nc.dram_tensor("out", [B, 1024], f32, kind="ExternalOutput").ap()
    d_dbg = {}
    if debug:
        for nm, shp in [("h2T", [128, BN_]), ("feat", [128, 64]),
                        ("FT", [64, 128]), ("capsT", [64, 128]),
                        ("lhsT", [128, 512]), ("u0", [128, 1024]),
                        ("u1", [128, 1024]), ("u2", [128, 1024]),
                        ("u3", [128, 1024]), ("blog0", [128, 128]),
                        ("sg0", [4, 1024]), ("a0", [4, 1024]),
                        ("c1it", [128, 128]), ("sg1", [4, 1024])]:
            d_dbg[nm] = nc.dram_tensor("dbg_" + nm, shp, f32,
                                       kind="ExternalOutput").ap()

    # collective bounce buffers (internal DRAM); A2A operates on first-dim
    # blocks: in[j] goes to rank j, out[r] came from rank r.
    d_a2a_in = [nc.dram_tensor(f"a2a_in_r{r}", [8, 2, B, 128], f32)
                for r in range(reps)]
    d_a2a_out = [nc.dram_tensor(f"a2a_out_r{r}", [8, 2, B, 128], f32)
                 for r in range(reps)]
    d_s_in = [[nc.dram_tensor(f"s_in{t}_r{r}", [B, 1024], f32)
               for t in range(3)] for r in range(reps)]
    d_s_out = [[nc.dram_tensor(f"s_out{t}_r{r}", [B, 1024], f32,
                               addr_space="Shared") for t in range(3)]
               for r in range(reps)]

    rg = [list(range(N_CORES))]

    with tile.TileContext(nc) as tc, ExitStack() as ctx:
        const = ctx.enter_context(tc.tile_pool(name="const", bufs=1))
        big = ctx.enter_context(tc.tile_pool(name="big", bufs=1))
        work = ctx.enter_context(tc.tile_pool(name="work", bufs=2))
        small = ctx.enter_context(tc.tile_pool(name="small", bufs=1))

        # ---- load constants / weights ----
        def load_const(name, dram, shape, dt, eng=None):
            t = const.tile(shape, dt, name=name)
            (eng or nc.sync).dma_start(out=t, in_=dram)
            return t

        xT = load_const("xT_sb", d_xT, [3, BN_], f32r)
        w1f = load_const("w1f_sb", d_w1f, [3, 64], f32r)
        c1f = load_const("c1f_sb", d_c1f, [64, 1], f32)
        w2f = load_const("w2f_sb", d_w2f, [64, 128], f32r)
        c2f = load_const("c2f_sb", d_c2f, [128, 1], f32)
        wbT = load_const("wbT_sb", d_wbT, [128, 2048], f32r)
        cb = load_const("cb_sb", d_cb, [128, 16], f32)
        sel132 = load_const("sel132_sb", d_sel132, [128, 4], bf16)
        sel1 = load_const("sel1_sb", d_sel1, [128, 4], bf16)
        sel4to128 = load_const("sel4to128_sb", d_sel4to128, [4, 128], f32r)
        selsq = load_const("selsq_sb", d_selsq, [64, 4], f32r)
        sel4to64 = load_const("sel4to64_sb", d_sel4to64, [4, 64], f32r)
        espread = load_const("espread_sb", d_espread, [64, 512], bf16)
        maskd = load_const("maskd_sb", d_maskd, [128, 512], bf16)

        ident = load_const("ident_sb", d_ident, [128, 128], f32)
        wc_all = const.tile([128, 16384], bf16, name="wc_all_sb")
        nc.sync.dma_start(
            out=wc_all.rearrange("p (g f) -> p g f", g=16),
            in_=d_wc.rearrange("g p f -> p g f"))
        wc_sb = [wc_all[:, 1024 * g:1024 * (g + 1)] for g in range(16)]

        def _body(rep):
            h2T = big.tile([128, BN_], f32r)  # [channel, point]

            # ---- phase A: shared MLP interleaved per-b with the branch
            # matmuls + fused maxpool: batch b's branch tiles only need
            # h2T chunks 4b..4b+3, so the MLP of b+1 hides under the
            # evacuation of b's tiles and the PE never drains.
            feat_sb = big.tile([128, 64], f32)  # [o_in_chunk, (k, oc, b)]
            feat_pt = big.tile([128, 256], f32)  # per-quarter partials
            with tc.tile_pool(name="ps_mlp", bufs=1, space="PSUM") as ps_mlp, \
                 tc.tile_pool(name="ps_y", bufs=3, space="PSUM") as ps_y:
                idx = 0
                for b in range(B):
                    for q in range(4):
                        sl = bass.ts(4 * b + q, 512)
                        p1 = ps_mlp.tile([64, 512], f32, tag="p1")
                        nc.tensor.matmul(p1, w1f, xT[:, sl], start=True,
                                         stop=True)
                        h1c = work.tile([64, 512], f32r, tag="h1c", bufs=4)
                        if q % 2 == 0:
                            nc.scalar.activation(out=h1c, in_=p1,
                                                 func=AF.Relu, bias=c1f,
                                                 scale=1.0)
                        else:
                            nc.vector.tensor_scalar(out=h1c, in0=p1,
                                                    scalar1=c1f, scalar2=0.0,
                                                    op0=AL.add, op1=AL.max)
                        p2 = ps_mlp.tile([128, 512], f32, tag="p2")
                        nc.tensor.matmul(p2, w2f, h1c, start=True, stop=True)
                        if q % 2 == 0:
                            nc.vector.tensor_scalar(out=h2T[:, sl], in0=p2,
                                                    scalar1=c2f, scalar2=0.0,
                                                    op0=AL.add, op1=AL.max)
                        else:
                            nc.scalar.activation(out=h2T[:, sl], in_=p2,
                                                 func=AF.Relu, bias=c2f,
                                                 scale=1.0)
                    if stage <= 1:
                        continue
                    for k in range(2):
                        for oc in range(8):
                            lw = wbT[:, bass.ts(k * 8 + oc, 128)]
                            for h in range(2):
                                py = ps_y.tile([128, 1024], f32, tag="py")
                                for q2 in range(2):
                                    q = 2 * h + q2
                                    nc.tensor.matmul(
                                        py[:, bass.ts(q2, 512)], lw,
                                        h2T[:, bass.ts(4 * b + q, 512)],
                                        start=True, stop=True)
                                # evacuate the two halves concurrently on
                                # two different engines so the psum tile
                                # frees in ~one half-evac time
                                for q2 in range(2):
                                    q = 2 * h + q2
                                    s3 = (((oc * 2 + k) * 4 + b) * 4 + q)
                                    acc = feat_pt[:, s3:s3 + 1]
                                    hp = py[:, bass.ts(q2, 512)]
                                    path = MAXPOOL_PATHS[idx %
                                                         len(MAXPOOL_PATHS)]
                                    idx += 1
                                    if path == "A":
                                        # ACT evac to bf16; DVE remax (4x)
                                        pair = work.tile([128, 512], bf16,
                                                         tag="pair", bufs=10)
                                        nc.scalar.copy(pair, hp)
                                        nc.vector.tensor_scalar(
                                            out=pair, in0=pair,
                                            scalar1=-3.0e38, scalar2=None,
                                            op0=AL.max, op1=AL.max,
                                            accum_out=acc)
                                    else:
                                        # DVE or Pool direct from psum
                                        eng = (nc.vector if path == "D"
                                               else nc.gpsimd)
                                        eng.tensor_scalar(
                                            out=hp, in0=hp, scalar1=-3.0e38,
                                            scalar2=None, op0=AL.max,
                                            op1=AL.max, accum_out=acc)

            if stage <= 1:
                nc.sync.dma_start(d_out, h2T[0:B, 0:1024].bitcast(f32))
                return
            # combine the four quarter-maxes per block
            nc.vector.tensor_reduce(
                feat_sb, feat_pt.rearrange("p (s q) -> p s q", q=4),
                axis=AX.X, op=AL.max)

            # feat += cb (cb[p, (oc, k)] broadcast over b)
            feat_v = feat_sb.rearrange("p (oc k b) -> p oc k b", oc=8, k=2)
            cb_bc = cb.rearrange("p (oc k) -> p oc k", oc=8).unsqueeze(3) \
                      .broadcast_to((128, 8, 2, 4))
            nc.vector.tensor_add(feat_v, feat_v, cb_bc)

            if debug and rep == 0:
                nc.sync.dma_start(d_dbg["feat"], feat_sb)
            if stage <= 2:
                nc.sync.dma_start(d_out[:, 0:32], feat_sb[0:B, 0:32])
                return
            # transpose feat on the PE so the a2a_in DMA is one contiguous
            # 32KB copy (featT flat layout == a2a_in flat layout).
            with tc.tile_pool(name="ps_ft", bufs=1, space="PSUM") as ps_ft:
                p_ftr = ps_ft.tile([64, 128], f32, tag="pft")
                nc.tensor.transpose(p_ftr, feat_sb, ident)
                featT = work.tile([64, 128], f32, tag="featT")
                nc.vector.tensor_copy(featT, p_ftr)
            nc.sync.dma_start(d_a2a_in[rep].ap(), featT)

            # ---- AllToAll: out viewed [16(e), B, 128(i_local)] ----
            if nocoll:
                nc.sync.dma_start(d_a2a_out[rep].ap(), d_a2a_in[rep].ap())
            else:
                nc.gpsimd.collective_compute(
                    "AllToAll", AL.bypass, ins=[d_a2a_in[rep].ap().opt()],
                    outs=[d_a2a_out[rep].ap().opt()], replica_groups=rg)

            # ---- phase B ----
            with tc.tile_pool(name="ps_b", bufs=2, space="PSUM") as ps_b, \
                 tc.tile_pool(name="ps_tiny", bufs=1, space="PSUM") as ps_tiny, \
                 tc.tile_pool(name="ps_s", bufs=1, space="PSUM") as ps_s:

                # caps: squash over branch axis e. FT[q = 4e+b, i_local]:
                # a2a_out flat row (r,k,b) = 4*(2r+k)+b = 4e+b, so the whole
                # tile is one contiguous 32KB DMA.
                FT = big.tile([64, 128], f32)
                nc.sync.dma_start(
                    FT, d_a2a_out[rep].ap().rearrange("r k b l -> (r k b) l"))

                FT2 = work.tile([64, 128], f32r, tag="ft2")
                nc.vector.tensor_mul(FT2, FT, FT)
                p_n2 = ps_tiny.tile([4, 128], f32, tag="pp")
                nc.tensor.matmul(p_n2, selsq, FT2,
                                 start=True, stop=True)
                # factor = sqrt(n2)/(1+n2) = exp(0.5*ln(n2) - ln(1+n2));
                # Ln/Exp/Relu/Copy/Square share one act table set, so no
                # act-table reloads anywhere in the kernel.
                l1 = small.tile([4, 128], f32, tag="l1")
                nc.scalar.activation(out=l1, in_=p_n2, func=AF.Ln, bias=0.0,
                                     scale=1.0)
                l2 = small.tile([4, 128], f32, tag="l2")
                nc.scalar.activation(out=l2, in_=p_n2, func=AF.Ln, bias=1.0,
                                     scale=1.0)
                ld = small.tile([4, 128], f32, tag="ld")
                nc.vector.scalar_tensor_tensor(
                    out=ld, in0=l1, scalar=0.5, in1=l2,
                    op0=AL.mult, op1=AL.subtract)
                fct = small.tile([4, 128], f32r, tag="fct")
                nc.scalar.activation(out=fct, in_=ld, func=AF.Exp, bias=0.0,
                                     scale=1.0)
                p_fbc = ps_tiny.tile([64, 128], f32, tag="pp")
                nc.tensor.matmul(p_fbc, sel4to64, fct,
                                 start=True, stop=True)
                capsT = work.tile([64, 128], bf16, tag="capsT")
                nc.vector.tensor_mul(capsT, FT, p_fbc)
                if debug and rep == 0:
                    nc.sync.dma_start(d_dbg["FT"], FT)
                    nc.gpsimd.dma_start(out=d_dbg["capsT"], in_=capsT)

                # block-diagonal lhsT[16j+e, 32g+4j'+b] = caps[b, 16j'+g, e]
                # * delta(j==j') via PE spread + masked mul (no DMAs):
                # OUT4[16j+e, 128b + i] = caps[b, i, e] (j-replicated).
                p_sp = ps_tiny.tile([128, 512], f32, tag="psp")
                for b in range(4):
                    nc.tensor.matmul(p_sp[:, bass.ts(b, 128)],
                                     espread[:, bass.ts(b, 128)], capsT,
                                     start=True, stop=True)
                lhsT = big.tile([128, 512], bf16)
                nc.vector.tensor_mul(
                    lhsT.rearrange("p (g j b) -> p g j b", g=16, j=8, b=4),
                    p_sp.rearrange("p (b j g) -> p g j b", b=4, j=8, g=16),
                    maskd.rearrange("p (g j b) -> p g j b", g=16, j=8, b=4))

                if debug and rep == 0:
                    nc.gpsimd.dma_start(out=d_dbg["lhsT"], in_=lhsT)
                # u matmuls: 16 groups of 8 capsules; 4 groups col-tiled
                # per psum tile. u_all[p = 32q + 4j + b, (t, o, v)] in bf16,
                # i_local = 16j + (4t+q)
                u_all = big.tile([128, 4096], bf16)
                u_evac = [nc.vector, nc.scalar, nc.gpsimd, nc.scalar]
                # ps0 accumulates the iteration-0 partial s0 = sum_i u/32;
                # its selector matmuls interleave with u production so the
                # first AllReduce can launch as soon as the last u tile
                # lands (accumulation groups on separate psum banks).
                ps0 = ps_s.tile([4, 1024], f32, tag="ps")
                for t in range(4):
                    pu = ps_b.tile([128, 1024], f32, tag="pu")
                    for q in range(4):
                        g = 4 * t + q
                        for h in range(2):
                            nc.tensor.matmul(
                                pu[32 * q:32 * q + 32, bass.ts(h, 512)],
                                lhsT[:, bass.ts(g, 32)],
                                wc_sb[g][:, bass.ts(h, 512)],
                                start=True, stop=True,
                                tile_position=(0, 32 * q))
                    if t == 1 or t == 3:
                        nc.scalar.copy(u_all[:, bass.ts(t, 1024)], pu)
                    else:
                        u_evac[t].tensor_copy(u_all[:, bass.ts(t, 1024)], pu)
                    for h in range(2):
                        nc.tensor.matmul(
                            ps0[:, bass.ts(h, 512)], sel132,
                            u_all[:, 1024 * t + 512 * h:
                                  1024 * t + 512 * (h + 1)],
                            start=(t == 0), stop=(t == 3),
                            skip_group_check=True)
                    if debug and rep == 0:
                        nc.gpsimd.dma_start(out=d_dbg[f"u{t}"],
                                            in_=u_all[:, bass.ts(t, 1024)])

                # ---- routing ----
                b_log = big.tile([128, 128], f32)  # [(q,j,b), (t,o)]
                uv = u_all.rearrange("p (t o v) -> p t o v", t=4, v=32)

                def s_round(c_sb, sel, pst):
                    # pst[4, 1024] = sum_t sel.T @ (u[:, t] * c_bc) — the
                    # weighted mul of tile t is interleaved with its two
                    # accumulating matmuls so PE overlaps DVE.
                    for t in range(4):
                        if c_sb is None:
                            wt_t = u_all[:, bass.ts(t, 1024)]
                        else:
                            wtt = work.tile([128, 1024], bf16, tag="wt",
                                            bufs=4)
                            nc.vector.tensor_mul(
                                wtt.rearrange("p (o v) -> p o v", v=32),
                                uv[:, t, :, :],
                                c_sb[:, bass.ts(t, 32)].unsqueeze(2)
                                    .broadcast_to((128, 32, 32)))
                            wt_t = wtt
                        for h in range(2):
                            nc.tensor.matmul(
                                pst[:, bass.ts(h, 512)], sel,
                                wt_t[:, bass.ts(h, 512)],
                                start=(t == 0), stop=(t == 3))

                def allreduce_s(pst, it):
                    s_loc = small.tile([4, 1024], f32, tag="s_loc")
                    nc.scalar.copy(s_loc, pst)
                    nc.sync.dma_start(d_s_in[rep][it].ap(), s_loc)
                    if nocoll:
                        nc.sync.dma_start(d_s_out[rep][it].ap(),
                                          d_s_in[rep][it].ap())
                    else:
                        nc.gpsimd.collective_compute(
                            "AllReduce", AL.add,
                            ins=[d_s_in[rep][it].ap().opt()],
                            outs=[d_s_out[rep][it].ap().opt()],
                            replica_groups=rg)
                    s_glob = small.tile([4, 1024], f32, tag=f"s_glob{it}")
                    nc.scalar.dma_start(out=s_glob, in_=d_s_out[rep][it].ap())
                    return s_glob

                def broadcast_s(s_glob):
                    # s_bc[p,(o,v)] = s_glob[b(p),(o,v)]; runs on PE in
                    # parallel with the squash-factor chain.
                    p_s = ps_b.tile([128, 1024], f32, tag="pu")
                    sgr = s_glob.bitcast(f32r)
                    for h in range(2):
                        nc.tensor.matmul(p_s[:, bass.ts(h, 512)], sel4to128,
                                         sgr[:, bass.ts(h, 512)],
                                         start=True, stop=True)
                    return p_s

                def squash_factor(s_glob):
                    # f[b,o] = |s|/(1+|s|^2) = exp(0.5*ln(n2) - ln(1+n2))
                    s2 = small.tile([4, 1024], f32, tag="sq_s2")
                    nc.scalar.square(s2, s_glob)
                    sn2 = small.tile([4, 32], f32, tag="sq_n2")
                    nc.vector.reduce_sum(
                        sn2, s2.rearrange("p (o v) -> p o v", v=32), axis=AX.X)
                    sl1 = small.tile([4, 32], f32, tag="sq_l1")
                    nc.scalar.activation(out=sl1, in_=sn2, func=AF.Ln,
                                         bias=0.0, scale=1.0)
                    sl2 = small.tile([4, 32], f32, tag="sq_l2")
                    nc.scalar.activation(out=sl2, in_=sn2, func=AF.Ln,
                                         bias=1.0, scale=1.0)
                    sld = small.tile([4, 32], f32, tag="sq_ld")
                    nc.vector.scalar_tensor_tensor(
                        out=sld, in0=sl1, scalar=0.5, in1=sl2,
                        op0=AL.mult, op1=AL.subtract)
                    sf = small.tile([4, 32], f32r, tag="sq_f")
                    nc.scalar.activation(out=sf, in_=sld, func=AF.Exp,
                                         bias=0.0, scale=1.0)
                    return sf

                def agree_update(s_bc, sf, first):
                    # abc[p,(o,v)] = squash(s)[b(p),(o,v)] broadcast:
                    # s_bc (done during the norm chain) times f_bc, fused.
                    p_f = ps_tiny.tile([128, 32], f32, tag="psp")
                    nc.tensor.matmul(p_f, sel4to128, sf, start=True, stop=True)
                    abc_sb = work.tile([128, 1024], bf16, tag="abc")
                    nc.vector.tensor_mul(
                        abc_sb.rearrange("p (o v) -> p o v", v=32),
                        s_bc.rearrange("p (o v) -> p o v", v=32),
                        p_f.unsqueeze(2).broadcast_to((128, 32, 32)))
                    abc_bc = abc_sb.rearrange("p (o v) -> p o v", v=32)
                    for t in range(4):
                        tmp = work.tile([128, 1024], bf16, tag="tmp")
                        tv = tmp.rearrange("p (o v) -> p o v", v=32)
                        nc.vector.tensor_mul(
                            tv, uv[:, t, :, :], abc_bc)
                        if first:
                            nc.vector.reduce_sum(
                                b_log[:, bass.ts(t, 32)], tv, axis=AX.X)
                        else:
                            agr = work.tile([128, 32], f32, tag="agr")
                            nc.vector.reduce_sum(agr, tv, axis=AX.X)
                            nc.vector.tensor_add(b_log[:, bass.ts(t, 32)],
                                                 b_log[:, bass.ts(t, 32)],
                                                 agr)

                def softmax_c():
                    cexp = work.tile([128, 128], f32, tag="cexp")
                    nc.scalar.activation(out=cexp, in_=b_log, func=AF.Exp,
                                         bias=0.0, scale=1.0)
                    sums = small.tile([128, 4], f32, tag="csum")
                    nc.vector.reduce_sum(
                        sums, cexp.rearrange("p (t o) -> p t o", o=32),
                        axis=AX.X)
                    crec = small.tile([128, 4], f32, tag="crec")
                    nc.vector.reciprocal(crec, sums)
                    c_sb = work.tile([128, 128], bf16, tag="c_sb")
                    nc.vector.tensor_mul(
                        c_sb.rearrange("p (t o) -> p t o", o=32),
                        cexp.rearrange("p (t o) -> p t o", o=32),
                        crec.unsqueeze(2).broadcast_to((128, 4, 32)))
                    return c_sb

                # iteration 0: ps0 was accumulated inside the u loop
                sg0 = allreduce_s(ps0, 0)
                if debug and rep == 0:
                    nc.sync.dma_start(d_dbg["sg0"], sg0)
                sbc0 = broadcast_s(sg0)
                agree_update(sbc0, squash_factor(sg0), first=True)
                if debug and rep == 0:
                    nc.sync.dma_start(d_dbg["blog0"], b_log)

                # iteration 1
                c1it = softmax_c()
                if debug and rep == 0:
                    nc.sync.dma_start(d_dbg["c1it"], c1it)
                ps1 = ps_s.tile([4, 1024], f32, tag="ps")
                s_round(c1it, sel1, ps1)
                sg1 = allreduce_s(ps1, 1)
                if debug and rep == 0:
                    nc.sync.dma_start(d_dbg["sg1"], sg1)
                sbc1 = broadcast_s(sg1)
                agree_update(sbc1, squash_factor(sg1), first=False)

                # iteration 2 (final): local partial s only; the host
                # all-reduces across cores and applies the final squash.
                ps2 = ps_s.tile([4, 1024], f32, tag="ps")
                s_round(softmax_c(), sel1, ps2)
                s_out2 = small.tile([4, 1024], f32, tag="s_out2")
                nc.scalar.copy(s_out2, ps2)
                nc.sync.dma_start(d_out, s_out2)

        for _rep in range(reps):
            _body(_rep)


    nc.compile()
    return nc


def _prepare_inputs(x, w1, g1, b1, m1, v1, w2, g2, b2, m2, v2,
                    wb, gb, bb, mb, vb, Wc):
    """Host-side: fold BN into weights, transpose/shard for the device."""
    fl = np.float32
    x = np.asarray(x, fl); w1 = np.asarray(w1, fl); w2 = np.asarray(w2, fl)
    wb = np.asarray(wb, fl); Wc = np.asarray(Wc, fl)
    g1, b1, m1, v1 = (np.asarray(a, fl) for a in (g1, b1, m1, v1))
    g2, b2, m2, v2 = (np.asarray(a, fl) for a in (g2, b2, m2, v2))
    gb, bb, mb, vb = (np.asarray(a, fl) for a in (gb, bb, mb, vb))

    s1 = g1 / np.sqrt(v1 + EPS)
    c1 = b1 - m1 * s1
    w1f = (w1 * s1[:, None]).T.copy()            # [3, 64]
    c1f = np.ascontiguousarray(c1[:, None])

    s2 = g2 / np.sqrt(v2 + EPS)
    c2 = b2 - m2 * s2
    w2f = (w2 * s2[:, None]).T.copy()            # [64, 128]
    c2f = np.ascontiguousarray(c2[:, None])

    sb = gb / np.sqrt(vb + EPS)                  # [16, 1024]
    wbp = wb * sb[:, :, None]                    # [16, 1024, 128]
    cbv = bb - mb * sb                           # [16, 1024]

    xT = np.ascontiguousarray(x.reshape(BN_, 3).T)  # [3, 8192]

    p = np.arange(128)
    sel1 = ((p[:, None] % 4) == np.arange(4)[None, :]).astype(fl)
    sel132 = sel1 / 32.0
    sel4to128 = np.ascontiguousarray(sel1.T)
    # FT rows are q = 4e + b, so the per-b selectors key on q % 4
    q64 = np.arange(64)
    selsq = ((q64[:, None] % 4) == np.arange(4)[None, :]).astype(fl)
    sel4to64 = np.ascontiguousarray(selsq.T)
    # espread[4e+b, 128b2 + 16j + e2] = (b==b2)&(e==e2) for all j:
    # spreads capsT rows (4e+b) onto partitions (16j+e) per b-block.
    espread = np.zeros((64, 512), fl)
    for e in range(16):
        for b in range(4):
            for j in range(8):
                espread[4 * e + b, 128 * b + 16 * j + e] = 1.0
    # maskd[16j+e, 32g+4j2+b] = (j == j2)
    maskd = np.zeros((128, 512), fl)
    for j in range(8):
        for e in range(16):
            maskd[16 * j + e, np.arange(16) * 32 + 4 * j + np.arange(4)[:, None]] = 1.0

    shared = {
        "xT": xT, "w1f": w1f, "c1f": c1f, "w2f": w2f, "c2f": c2f,
        "sel132": sel132.astype(_BF), "sel1": sel1.astype(_BF),
        "sel4to128": sel4to128,
        "selsq": selsq, "sel4to64": sel4to64,
        "espread": espread.astype(_BF), "maskd": maskd.astype(_BF),
        "ident128": np.eye(128, dtype=fl),
    }

    in_maps = []
    for c in range(N_CORES):
        m = dict(shared)
        ks = slice(2 * c, 2 * c + 2)
        # wbT[p=ch, (k, oc, o)] = wbp[2c+k, 128*oc+o, ch]
        m["wbT"] = np.ascontiguousarray(
            wbp[ks].reshape(2, 8, 128, 128).transpose(3, 0, 1, 2)
            .reshape(128, 2048))
        # cb[p, (oc, k)] = cbv[2c+k, 128*oc+p]
        m["cb"] = np.ascontiguousarray(
            cbv[ks].reshape(2, 8, 128).transpose(2, 1, 0).reshape(128, 16))
        # wc[g, 16j+e, 32o+v] = Wc[o, 128c + 16j + g, e, v]  (i_local = 16j+g)
        wcs = Wc[:, 128 * c:128 * (c + 1)]       # [32, 128, 16, 32]
        m["wc"] = np.ascontiguousarray(
            wcs.reshape(32, 8, 16, 16, 32)       # [o, j, g, e, v]
            .transpose(2, 1, 3, 0, 4)            # [g, j, e, o, v]
            .reshape(16, 128, 1024)).astype(_BF)
        in_maps.append(m)
    return in_maps


def host_finish(parts):
    """Sum per-core routing partials s2[b,1024] and apply the final squash."""
    s = np.sum([np.asarray(p, dtype=np.float32) for p in parts], axis=0)
    s = s.reshape(B, 32, 32).astype(np.float64)
    n = np.linalg.norm(s, axis=2, keepdims=True)
    return (s * (n / (1.0 + n * n))).astype(np.float32)


def kernel(**inputs):
    if "nc" not in _CACHE:
        _CACHE["nc"] = _build_bass()
    nc = _CACHE["nc"]
    in_maps = _prepare_inputs(**inputs)
    res = bass_utils.run_bass_kernel_spmd(
        nc, in_maps, core_ids=list(range(N_CORES)))
    return host_finish([r["out"] for r in res.results])



# revision 27
# speedup vs baseline: 1.0112x; 1.0112x over previous
"""Trainium2 Bass kernel for nn_Encoder_55688545960036.

Network: pointnet-style shared MLP (3->64->128, eval-mode BN folded into the
weights on the host, relu), 16 branch matmuls 128->1024 with folded BN and a
maxpool over the 2048 points of each batch element, squash over the branch
axis, capsule transform u[b,o,i,v] = sum_e caps[b,i,e] * Wc[o,i,e,v], 3 rounds
of dynamic routing, squash -> [4,32,32] output.

Distribution over 8 NeuronCores:
  phase A: branch axis k=16 -> 2 branches per core (shared MLP replicated).
  phase B: in-capsule axis i=1024 -> 128 per core (Wc 67MB -> 8.4MB/core).
  collectives: one AllToAll that converts per-core (2 branches, all 1024 i)
  feat into per-core (all 16 branches, 128-i shard), then 3 AllReduces of the
  routing partial sums s[4,32,32] (one per routing iteration).

All heavy matmuls run in float32r (full PE rate, ~1.5e-4 rel err on HW).
The maxpool is fused into PSUM evacuation with tensor_scalar accum max.
The capsule einsum is done as 16 block-diagonal matmuls (8 capsules of the
i-shard per group, caps entries on the block diagonal) with PE column tiling.
Routing keeps (i,b) on partitions and (o,v) in the free dims so the softmax
over o is a free-axis op; sums over i (partitions) are tiny selector matmuls.
"""

import functools

import numpy as np
from contextlib import ExitStack

import concourse.bass as bass
import concourse.tile as tile
from concourse import bacc, mybir
from concourse import bass_utils

# All ACT functions this kernel uses (Relu/Copy/Square/Ln/Exp/Identity) live
# together in the "natural_log_exp_and_others" table set, but the act-table
# placement pass greedily picks the FIRST set containing each func, which
# ping-pongs between Ln-only and Exp-only sets (one 1.3us table load per
# switch). Empty every other set (preserving dict order so walrus's
# act_func_set_id mapping is unchanged) so all activations resolve to the one
# set and a single load suffices.
_KEEP_ACT_SET = "natural_log_exp_and_others"
_orig_gat = bacc.get_activation_tables


@functools.cache
def _patched_gat(arch):
    t = _orig_gat(arch)
    return {k: (v if k == _KEEP_ACT_SET else set()) for k, v in t.items()}


bacc.get_activation_tables = _patched_gat

EPS = 1e-5
N_CORES = 8
B = 4
BN_ = 4 * 2048  # 8192 points
f32 = mybir.dt.float32
f32r = mybir.dt.float32r
AL = mybir.AluOpType
AF = mybir.ActivationFunctionType
AX = mybir.AxisListType
bf16 = mybir.dt.bfloat16
_BF = mybir.dt.np(bf16)

_CACHE = {}

# maxpool evacuation paths: D = DVE-direct-from-psum, A = ACT-copy +
# DVE remax (bf16, 4x), P = Pool(GpSimd)-direct-from-psum. Weighted
# round-robin so DVE/ACT/Pool busy times balance against the PE.
_PATH_W = {"A": 7, "D": 4, "P": 5}


def _make_paths(n=128):
    paths, cnt = [], {k: 0 for k in _PATH_W}
    for i in range(n):
        pick = max(_PATH_W, key=lambda k: _PATH_W[k] * (i + 1) / 16 - cnt[k])
        paths.append(pick)
        cnt[pick] += 1
    return paths


MAXPOOL_PATHS = _make_paths()


def _build_bass(reps=1, debug=False, stage=4, nocoll=False):
    # stage: 1=MLP only, 2=+branch/maxpool, 3=+A2A/caps/u, 4=full (routing)
    # nocoll: replace collectives with local DRAM copies (for TimelineSim)
    # reps > 1 replicates the compute body end-to-end inside one NEFF; used
    # only for wall-clock-difference timing in the dev harness.
    nc = bacc.Bacc("TRN2", target_bir_lowering=False, debug=False,
                   num_devices=N_CORES)

    # ---- DRAM I/O ----
    d_xT = nc.dram_tensor("xT", [3, BN_], f32r, kind="ExternalInput").ap()
    d_w1f = nc.dram_tensor("w1f", [3, 64], f32r, kind="ExternalInput").ap()
    d_c1f = nc.dram_tensor("c1f", [64, 1], f32, kind="ExternalInput").ap()
    d_w2f = nc.dram_tensor("w2f", [64, 128], f32r, kind="ExternalInput").ap()
    d_c2f = nc.dram_tensor("c2f", [128, 1], f32, kind="ExternalInput").ap()
    d_wbT = nc.dram_tensor("wbT", [128, 2048], f32r, kind="ExternalInput").ap()
    d_cb = nc.dram_tensor("cb", [128, 16], f32, kind="ExternalInput").ap()
    d_wc = nc.dram_tensor("wc", [16, 128, 1024], bf16, kind="ExternalInput").ap()
    d_sel132 = nc.dram_tensor("sel132", [128, 4], bf16, kind="ExternalInput").ap()
    d_sel1 = nc.dram_tensor("sel1", [128, 4], bf16, kind="ExternalInput").ap()
    d_sel4to128 = nc.dram_tensor("sel4to128", [4, 128], f32r,
                                 kind="ExternalInput").ap()
    d_selsq = nc.dram_tensor("selsq", [64, 4], f32r, kind="ExternalInput").ap()
    d_sel4to64 = nc.dram_tensor("sel4to64", [4, 64], f32r,
                                kind="ExternalInput").ap()
    d_espread = nc.dram_tensor("espread", [64, 512], bf16,
                               kind="ExternalInput").ap()
    d_maskd = nc.dram_tensor("maskd", [128, 512], bf16,
                             kind="ExternalInput").ap()
    d_ident = nc.dram_tensor("ident128", [128, 128], f32,
                             kind="ExternalInput").ap()
    # "out" holds this core's iteration-2 routing partial s2[b, (o,v)];
    # the host sums across cores and applies the final squash.
    d_out = nc.dram_tensor("out", [B, 1024], f32, kind="ExternalOutput").ap()
    d_dbg = {}
    if debug:
        for nm, shp in [("h2T", [128, BN_]), ("feat", [128, 64]),
                        ("FT", [64, 128]), ("capsT", [64, 128]),
                        ("lhsT", [128, 512]), ("u0", [128, 1024]),
                        ("u1", [128, 1024]), ("u2", [128, 1024]),
                        ("u3", [128, 1024]), ("blog0", [128, 128]),
                        ("sg0", [4, 1024]), ("a0", [4, 1024]),
                        ("c1it", [128, 128]), ("sg1", [4, 1024])]:
            d_dbg[nm] = nc.dram_tensor("dbg_" + nm, shp, f32,
                                       kind="ExternalOutput").ap()

    # collective bounce buffers (internal DRAM); A2A operates on first-dim
    # blocks: in[j] goes to rank j, out[r] came from rank r.
    d_a2a_in = [nc.dram_tensor(f"a2a_in_r{r}", [8, 2, B, 128], f32)
                for r in range(reps)]
    d_a2a_out = [nc.dram_tensor(f"a2a_out_r{r}", [8, 2, B, 128], f32)
                 for r in range(reps)]
    d_s_in = [[nc.dram_tensor(f"s_in{t}_r{r}", [B, 1024], f32)
               for t in range(3)] for r in range(reps)]
    d_s_out = [[nc.dram_tensor(f"s_out{t}_r{r}", [B, 1024], f32,
                               addr_space="Shared") for t in range(3)]
               for r in range(reps)]

    rg = [list(range(N_CORES))]

    with tile.TileContext(nc) as tc, ExitStack() as ctx:
        const = ctx.enter_context(tc.tile_pool(name="const", bufs=1))
        big = ctx.enter_context(tc.tile_pool(name="big", bufs=1))
        work = ctx.enter_context(tc.tile_pool(name="work", bufs=2))
        small = ctx.enter_context(tc.tile_pool(name="small", bufs=1))

        # ---- load constants / weights ----
        def load_const(name, dram, shape, dt, eng=None):
            t = const.tile(shape, dt, name=name)
            (eng or nc.sync).dma_start(out=t, in_=dram)
            return t

        xT = load_const("xT_sb", d_xT, [3, BN_], f32r)
        w1f = load_const("w1f_sb", d_w1f, [3, 64], f32r)
        c1f = load_const("c1f_sb", d_c1f, [64, 1], f32)
        w2f = load_const("w2f_sb", d_w2f, [64, 128], f32r)
        c2f = load_const("c2f_sb", d_c2f, [128, 1], f32)
        wbT = load_const("wbT_sb", d_wbT, [128, 2048], f32r)
        cb = load_const("cb_sb", d_cb, [128, 16], f32)
        sel132 = load_const("sel132_sb", d_sel132, [128, 4], bf16)
        sel1 = load_const("sel1_sb", d_sel1, [128, 4], bf16)
        sel4to128 = load_const("sel4to128_sb", d_sel4to128, [4, 128], f32r)
        selsq = load_const("selsq_sb", d_selsq, [64, 4], f32r)
        sel4to64 = load_const("sel4to64_sb", d_sel4to64, [4, 64], f32r)
        espread = load_const("espread_sb", d_espread, [64, 512], bf16)
        maskd = load_const("maskd_sb", d_maskd, [128, 512], bf16)

        ident = load_const("ident_sb", d_ident, [128, 128], f32)
        wc_all = const.tile([128, 16384], bf16, name="wc_all_sb")
        nc.sync.dma_start(
            out=wc_all.rearrange("p (g f) -> p g f", g=16),
            in_=d_wc.rearrange("g p f -> p g f"))
        wc_sb = [wc_all[:, 1024 * g:1024 * (g + 1)] for g in range(16)]

        def _body(rep):
            h2T = big.tile([128, BN_], f32r)  # [channel, point]

            # ---- phase A: shared MLP interleaved per-b with the branch
            # matmuls + fused maxpool: batch b's branch tiles only need
            # h2T chunks 4b..4b+3, so the MLP of b+1 hides under the
            # evacuation of b's tiles and the PE never drains.
            feat_sb = big.tile([128, 64], f32)  # [o_in_chunk, (k, oc, b)]
            feat_pt = big.tile([128, 256], f32)  # per-quarter partials
            with tc.tile_pool(name="ps_mlp", bufs=1, space="PSUM") as ps_mlp, \
                 tc.tile_pool(name="ps_y", bufs=3, space="PSUM") as ps_y:
                idx = 0
                for b in range(B):
                    for q in range(4):
                        sl = bass.ts(4 * b + q, 512)
                        p1 = ps_mlp.tile([64, 512], f32, tag="p1")
                        nc.tensor.matmul(p1, w1f, xT[:, sl], start=True,
                                         stop=True)
                        h1c = work.tile([64, 512], f32r, tag="h1c", bufs=4)
                        if q % 2 == 0:
                            nc.scalar.activation(out=h1c, in_=p1,
                                                 func=AF.Relu, bias=c1f,
                                                 scale=1.0)
                        else:
                            nc.vector.tensor_scalar(out=h1c, in0=p1,
                                                    scalar1=c1f, scalar2=0.0,
                                                    op0=AL.add, op1=AL.max)
                        p2 = ps_mlp.tile([128, 512], f32, tag="p2")
                        nc.tensor.matmul(p2, w2f, h1c, start=True, stop=True)
                        if q % 2 == 0:
                            nc.vector.tensor_scalar(out=h2T[:, sl], in0=p2,
                                                    scalar1=c2f, scalar2=0.0,
                                                    op0=AL.add, op1=AL.max)
                        else:
                            nc.scalar.activation(out=h2T[:, sl], in_=p2,
                                                 func=AF.Relu, bias=c2f,
                                                 scale=1.0)
                    if stage <= 1:
                        continue
                    for k in range(2):
                        for oc in range(8):
                            lw = wbT[:, bass.ts(k * 8 + oc, 128)]
                            for h in range(2):
                                py = ps_y.tile([128, 1024], f32, tag="py")
                                for q2 in range(2):
                                    q = 2 * h + q2
                                    nc.tensor.matmul(
                                        py[:, bass.ts(q2, 512)], lw,
                                        h2T[:, bass.ts(4 * b + q, 512)],
                                        start=True, stop=True)
                                # evacuate the two halves concurrently on
                                # two different engines so the psum tile
                                # frees in ~one half-evac time
                                for q2 in range(2):
                                    q = 2 * h + q2
                                    s3 = (((oc * 2 + k) * 4 + b) * 4 + q)
                                    acc = feat_pt[:, s3:s3 + 1]
                                    hp = py[:, bass.ts(q2, 512)]
                                    path = MAXPOOL_PATHS[idx %
                                                         len(MAXPOOL_PATHS)]
                                    idx += 1
                                    if path == "A":
                                        # ACT evac to bf16; DVE remax (4x)
                                        pair = work.tile([128, 512], bf16,
                                                         tag="pair", bufs=10)
                                        nc.scalar.copy(pair, hp)
                                        nc.vector.tensor_scalar(
                                            out=pair, in0=pair,
                                            scalar1=-3.0e38, scalar2=None,
                                            op0=AL.max, op1=AL.max,
                                            accum_out=acc)
                                    else:
                                        # DVE or Pool direct from psum; the
                                        # ALU out goes to SBUF scratch so
                                        # the psum tile is only read (both
                                        # halves evacuate in parallel)
                                        eng = (nc.vector if path == "D"
                                               else nc.gpsimd)
                                        junk = work.tile(
                                            [128, 512], f32,
                                            tag="junk" + path, bufs=2)
                                        eng.tensor_scalar(
                                            out=junk, in0=hp, scalar1=-3.0e38,
                                            scalar2=None, op0=AL.max,
                                            op1=AL.max, accum_out=acc)

            if stage <= 1:
                nc.sync.dma_start(d_out, h2T[0:B, 0:1024].bitcast(f32))
                return
            # combine the four quarter-maxes per block
            nc.vector.tensor_reduce(
                feat_sb, feat_pt.rearrange("p (s q) -> p s q", q=4),
                axis=AX.X, op=AL.max)

            # feat += cb (cb[p, (oc, k)] broadcast over b)
            feat_v = feat_sb.rearrange("p (oc k b) -> p oc k b", oc=8, k=2)
            cb_bc = cb.rearrange("p (oc k) -> p oc k", oc=8).unsqueeze(3) \
                      .broadcast_to((128, 8, 2, 4))
            nc.vector.tensor_add(feat_v, feat_v, cb_bc)

            if debug and rep == 0:
                nc.sync.dma_start(d_dbg["feat"], feat_sb)
            if stage <= 2:
                nc.sync.dma_start(d_out[:, 0:32], feat_sb[0:B, 0:32])
                return
            # transpose feat on the PE so the a2a_in DMA is one contiguous
            # 32KB copy (featT flat layout == a2a_in flat layout).
            with tc.tile_pool(name="ps_ft", bufs=1, space="PSUM") as ps_ft:
                p_ftr = ps_ft.tile([64, 128], f32, tag="pft")
                nc.tensor.transpose(p_ftr, feat_sb, ident)
                featT = work.tile([64, 128], f32, tag="featT")
                nc.vector.tensor_copy(featT, p_ftr)
            nc.sync.dma_start(d_a2a_in[rep].ap(), featT)

            # ---- AllToAll: out viewed [16(e), B, 128(i_local)] ----
            if nocoll:
                nc.sync.dma_start(d_a2a_out[rep].ap(), d_a2a_in[rep].ap())
            else:
                nc.gpsimd.collective_compute(
                    "AllToAll", AL.bypass, ins=[d_a2a_in[rep].ap().opt()],
                    outs=[d_a2a_out[rep].ap().opt()], replica_groups=rg)

            # ---- phase B ----
            with tc.tile_pool(name="ps_b", bufs=2, space="PSUM") as ps_b, \
                 tc.tile_pool(name="ps_tiny", bufs=1, space="PSUM") as ps_tiny, \
                 tc.tile_pool(name="ps_s", bufs=1, space="PSUM") as ps_s:

                # caps: squash over branch axis e. FT[q = 4e+b, i_local]:
                # a2a_out flat row (r,k,b) = 4*(2r+k)+b = 4e+b, so the whole
                # tile is one contiguous 32KB DMA.
                FT = big.tile([64, 128], f32)
                nc.sync.dma_start(
                    FT, d_a2a_out[rep].ap().rearrange("r k b l -> (r k b) l"))

                FT2 = work.tile([64, 128], f32r, tag="ft2")
                nc.vector.tensor_mul(FT2, FT, FT)
                p_n2 = ps_tiny.tile([4, 128], f32, tag="pp")
                nc.tensor.matmul(p_n2, selsq, FT2,
                                 start=True, stop=True)
                # factor = sqrt(n2)/(1+n2) = exp(0.5*ln(n2) - ln(1+n2));
                # Ln/Exp/Relu/Copy/Square share one act table set, so no
                # act-table reloads anywhere in the kernel.
                l1 = small.tile([4, 128], f32, tag="l1")
                nc.scalar.activation(out=l1, in_=p_n2, func=AF.Ln, bias=0.0,
                                     scale=1.0)
                l2 = small.tile([4, 128], f32, tag="l2")
                nc.scalar.activation(out=l2, in_=p_n2, func=AF.Ln, bias=1.0,
                                     scale=1.0)
                ld = small.tile([4, 128], f32, tag="ld")
                nc.vector.scalar_tensor_tensor(
                    out=ld, in0=l1, scalar=0.5, in1=l2,
                    op0=AL.mult, op1=AL.subtract)
                fct = small.tile([4, 128], f32r, tag="fct")
                nc.scalar.activation(out=fct, in_=ld, func=AF.Exp, bias=0.0,
                                     scale=1.0)
                p_fbc = ps_tiny.tile([64, 128], f32, tag="pp")
                nc.tensor.matmul(p_fbc, sel4to64, fct,
                                 start=True, stop=True)
                capsT = work.tile([64, 128], bf16, tag="capsT")
                nc.vector.tensor_mul(capsT, FT, p_fbc)
                if debug and rep == 0:
                    nc.sync.dma_start(d_dbg["FT"], FT)
                    nc.gpsimd.dma_start(out=d_dbg["capsT"], in_=capsT)

                # block-diagonal lhsT[16j+e, 32g+4j'+b] = caps[b, 16j'+g, e]
                # * delta(j==j') via PE spread + masked mul (no DMAs):
                # OUT4[16j+e, 128b + i] = caps[b, i, e] (j-replicated).
                p_sp = ps_tiny.tile([128, 512], f32, tag="psp")
                for b in range(4):
                    nc.tensor.matmul(p_sp[:, bass.ts(b, 128)],
                                     espread[:, bass.ts(b, 128)], capsT,
                                     start=True, stop=True)
                lhsT = big.tile([128, 512], bf16)
                nc.vector.tensor_mul(
                    lhsT.rearrange("p (g j b) -> p g j b", g=16, j=8, b=4),
                    p_sp.rearrange("p (b j g) -> p g j b", b=4, j=8, g=16),
                    maskd.rearrange("p (g j b) -> p g j b", g=16, j=8, b=4))

                if debug and rep == 0:
                    nc.gpsimd.dma_start(out=d_dbg["lhsT"], in_=lhsT)
                # u matmuls: 16 groups of 8 capsules; 4 groups col-tiled
                # per psum tile. u_all[p = 32q + 4j + b, (t, o, v)] in bf16,
                # i_local = 16j + (4t+q)
                u_all = big.tile([128, 4096], bf16)
                u_evac = [nc.vector, nc.scalar, nc.gpsimd, nc.scalar]
                # ps0 accumulates the iteration-0 partial s0 = sum_i u/32;
                # its selector matmuls interleave with u production so the
                # first AllReduce can launch as soon as the last u tile
                # lands (accumulation groups on separate psum banks).
                ps0 = ps_s.tile([4, 1024], f32, tag="ps")
                for t in range(4):
                    pu = ps_b.tile([128, 1024], f32, tag="pu")
                    for q in range(4):
                        g = 4 * t + q
                        for h in range(2):
                            nc.tensor.matmul(
                                pu[32 * q:32 * q + 32, bass.ts(h, 512)],
                                lhsT[:, bass.ts(g, 32)],
                                wc_sb[g][:, bass.ts(h, 512)],
                                start=True, stop=True,
                                tile_position=(0, 32 * q))
                    if t == 1 or t == 3:
                        nc.scalar.copy(u_all[:, bass.ts(t, 1024)], pu)
                    else:
                        u_evac[t].tensor_copy(u_all[:, bass.ts(t, 1024)], pu)
                    for h in range(2):
                        nc.tensor.matmul(
                            ps0[:, bass.ts(h, 512)], sel132,
                            u_all[:, 1024 * t + 512 * h:
                                  1024 * t + 512 * (h + 1)],
                            start=(t == 0), stop=(t == 3),
                            skip_group_check=True)
                    if debug and rep == 0:
                        nc.gpsimd.dma_start(out=d_dbg[f"u{t}"],
                                            in_=u_all[:, bass.ts(t, 1024)])

                # ---- routing ----
                b_log = big.tile([128, 128], f32)  # [(q,j,b), (t,o)]
                uv = u_all.rearrange("p (t o v) -> p t o v", t=4, v=32)

                def s_round(c_sb, sel, pst):
                    # pst[4, 1024] = sum_t sel.T @ (u[:, t] * c_bc) — the
                    # weighted mul of tile t is interleaved with its two
                    # accumulating matmuls so PE overlaps DVE.
                    for t in range(4):
                        if c_sb is None:
                            wt_t = u_all[:, bass.ts(t, 1024)]
                        else:
                            wtt = work.tile([128, 1024], bf16, tag="wt",
                                            bufs=4)
                            nc.vector.tensor_mul(
                                wtt.rearrange("p (o v) -> p o v", v=32),
                                uv[:, t, :, :],
                                c_sb[:, bass.ts(t, 32)].unsqueeze(2)
                                    .broadcast_to((128, 32, 32)))
                            wt_t = wtt
                        for h in range(2):
                            nc.tensor.matmul(
                                pst[:, bass.ts(h, 512)], sel,
                                wt_t[:, bass.ts(h, 512)],
                                start=(t == 0), stop=(t == 3))

                def allreduce_s(pst, it):
                    s_loc = small.tile([4, 1024], f32, tag="s_loc")
                    nc.scalar.copy(s_loc, pst)
                    nc.sync.dma_start(d_s_in[rep][it].ap(), s_loc)
                    if nocoll:
                        nc.sync.dma_start(d_s_out[rep][it].ap(),
                                          d_s_in[rep][it].ap())
                    else:
                        nc.gpsimd.collective_compute(
                            "AllReduce", AL.add,
                            ins=[d_s_in[rep][it].ap().opt()],
                            outs=[d_s_out[rep][it].ap().opt()],
                            replica_groups=rg)
                    s_glob = small.tile([4, 1024], f32, tag=f"s_glob{it}")
                    nc.scalar.dma_start(out=s_glob, in_=d_s_out[rep][it].ap())
                    return s_glob

                def broadcast_s(s_glob):
                    # s_bc[p,(o,v)] = s_glob[b(p),(o,v)]; runs on PE in
                    # parallel with the squash-factor chain.
                    p_s = ps_b.tile([128, 1024], f32, tag="pu")
                    sgr = s_glob.bitcast(f32r)
                    for h in range(2):
                        nc.tensor.matmul(p_s[:, bass.ts(h, 512)], sel4to128,
                                         sgr[:, bass.ts(h, 512)],
                                         start=True, stop=True)
                    return p_s

                def squash_factor(s_glob):
                    # f[b,o] = |s|/(1+|s|^2) = exp(0.5*ln(n2) - ln(1+n2))
                    s2 = small.tile([4, 1024], f32, tag="sq_s2")
                    nc.scalar.square(s2, s_glob)
                    sn2 = small.tile([4, 32], f32, tag="sq_n2")
                    nc.vector.reduce_sum(
                        sn2, s2.rearrange("p (o v) -> p o v", v=32), axis=AX.X)
                    sl1 = small.tile([4, 32], f32, tag="sq_l1")
                    nc.scalar.activation(out=sl1, in_=sn2, func=AF.Ln,
                                         bias=0.0, scale=1.0)
                    sl2 = small.tile([4, 32], f32, tag="sq_l2")
                    nc.scalar.activation(out=sl2, in_=sn2, func=AF.Ln,
                                         bias=1.0, scale=1.0)
                    sld = small.tile([4, 32], f32, tag="sq_ld")
                    nc.vector.scalar_tensor_tensor(
                        out=sld, in0=sl1, scalar=0.5, in1=sl2,
                        op0=AL.mult, op1=AL.subtract)
                    sf = small.tile([4, 32], f32r, tag="sq_f")
                    nc.scalar.activation(out=sf, in_=sld, func=AF.Exp,
                                         bias=0.0, scale=1.0)
                    return sf

                def agree_update(s_bc, sf, first):
                    # abc[p,(o,v)] = squash(s)[b(p),(o,v)] broadcast:
                    # s_bc (done during the norm chain) times f_bc, fused.
                    p_f = ps_tiny.tile([128, 32], f32, tag="psp")
                    nc.tensor.matmul(p_f, sel4to128, sf, start=True, stop=True)
                    abc_sb = work.tile([128, 1024], bf16, tag="abc")
                    nc.vector.tensor_mul(
                        abc_sb.rearrange("p (o v) -> p o v", v=32),
                        s_bc.rearrange("p (o v) -> p o v", v=32),
                        p_f.unsqueeze(2).broadcast_to((128, 32, 32)))
                    abc_bc = abc_sb.rearrange("p (o v) -> p o v", v=32)
                    for t in range(4):
                        tmp = work.tile([128, 1024], bf16, tag="tmp")
                        tv = tmp.rearrange("p (o v) -> p o v", v=32)
                        nc.vector.tensor_mul(
                            tv, uv[:, t, :, :], abc_bc)
                        if first:
                            nc.vector.reduce_sum(
                                b_log[:, bass.ts(t, 32)], tv, axis=AX.X)
                        else:
                            agr = work.tile([128, 32], f32, tag="agr")
                            nc.vector.reduce_sum(agr, tv, axis=AX.X)
                            nc.vector.tensor_add(b_log[:, bass.ts(t, 32)],
                                                 b_log[:, bass.ts(t, 32)],
                                                 agr)

                def softmax_c():
                    cexp = work.tile([128, 128], f32, tag="cexp")
                    nc.scalar.activation(out=cexp, in_=b_log, func=AF.Exp,
                                         bias=0.0, scale=1.0)
                    sums = small.tile([128, 4], f32, tag="csum")
                    nc.vector.reduce_sum(
                        sums, cexp.rearrange("p (t o) -> p t o", o=32),
                        axis=AX.X)
                    crec = small.tile([128, 4], f32, tag="crec")
                    nc.vector.reciprocal(crec, sums)
                    c_sb = work.tile([128, 128], bf16, tag="c_sb")
                    nc.vector.tensor_mul(
                        c_sb.rearrange("p (t o) -> p t o", o=32),
                        cexp.rearrange("p (t o) -> p t o", o=32),
                        crec.unsqueeze(2).broadcast_to((128, 4, 32)))
                    return c_sb

                # iteration 0: ps0 was accumulated inside the u loop
                sg0 = allreduce_s(ps0, 0)
                if debug and rep == 0:
                    nc.sync.dma_start(d_dbg["sg0"], sg0)
                sbc0 = broadcast_s(sg0)
                agree_update(sbc0, squash_factor(sg0), first=True)
                if debug and rep == 0:
                    nc.sync.dma_start(d_dbg["blog0"], b_log)

                # iteration 1
                c1it = softmax_c()
                if debug and rep == 0:
                    nc.sync.dma_start(d_dbg["c1it"], c1it)
                ps1 = ps_s.tile([4, 1024], f32, tag="ps")
                s_round(c1it, sel1, ps1)
                sg1 = allreduce_s(ps1, 1)
                if debug and rep == 0:
                    nc.sync.dma_start(d_dbg["sg1"], sg1)
                sbc1 = broadcast_s(sg1)
                agree_update(sbc1, squash_factor(sg1), first=False)

                # iteration 2 (final): local partial s only; the host
                # all-reduces across cores and applies the final squash.
                ps2 = ps_s.tile([4, 1024], f32, tag="ps")
                s_round(softmax_c(), sel1, ps2)
                s_out2 = small.tile([4, 1024], f32, tag="s_out2")
                nc.scalar.copy(s_out2, ps2)
                nc.sync.dma_start(d_out, s_out2)

        for _rep in range(reps):
            _body(_rep)


    nc.compile()
    return nc


def _prepare_inputs(x, w1, g1, b1, m1, v1, w2, g2, b2, m2, v2,
                    wb, gb, bb, mb, vb, Wc):
    """Host-side: fold BN into weights, transpose/shard for the device."""
    fl = np.float32
    x = np.asarray(x, fl); w1 = np.asarray(w1, fl); w2 = np.asarray(w2, fl)
    wb = np.asarray(wb, fl); Wc = np.asarray(Wc, fl)
    g1, b1, m1, v1 = (np.asarray(a, fl) for a in (g1, b1, m1, v1))
    g2, b2, m2, v2 = (np.asarray(a, fl) for a in (g2, b2, m2, v2))
    gb, bb, mb, vb = (np.asarray(a, fl) for a in (gb, bb, mb, vb))

    s1 = g1 / np.sqrt(v1 + EPS)
    c1 = b1 - m1 * s1
    w1f = (w1 * s1[:, None]).T.copy()            # [3, 64]
    c1f = np.ascontiguousarray(c1[:, None])

    s2 = g2 / np.sqrt(v2 + EPS)
    c2 = b2 - m2 * s2
    w2f = (w2 * s2[:, None]).T.copy()            # [64, 128]
    c2f = np.ascontiguousarray(c2[:, None])

    sb = gb / np.sqrt(vb + EPS)                  # [16, 1024]
    wbp = wb * sb[:, :, None]                    # [16, 1024, 128]
    cbv = bb - mb * sb                           # [16, 1024]

    xT = np.ascontiguousarray(x.reshape(BN_, 3).T)  # [3, 8192]

    p = np.arange(128)
    sel1 = ((p[:, None] % 4) == np.arange(4)[None, :]).astype(fl)
    sel132 = sel1 / 32.0
    sel4to128 = np.ascontiguousarray(sel1.T)
    # FT rows are q = 4e + b, so the per-b selectors key on q % 4
    q64 = np.arange(64)
    selsq = ((q64[:, None] % 4) == np.arange(4)[None, :]).astype(fl)
    sel4to64 = np.ascontiguousarray(selsq.T)
    # espread[4e+b, 128b2 + 16j + e2] = (b==b2)&(e==e2) for all j:
    # spreads capsT rows (4e+b) onto partitions (16j+e) per b-block.
    espread = np.zeros((64, 512), fl)
    for e in range(16):
        for b in range(4):
            for j in range(8):
                espread[4 * e + b, 128 * b + 16 * j + e] = 1.0
    # maskd[16j+e, 32g+4j2+b] = (j == j2)
    maskd = np.zeros((128, 512), fl)
    for j in range(8):
        for e in range(16):
            maskd[16 * j + e, np.arange(16) * 32 + 4 * j + np.arange(4)[:, None]] = 1.0

    shared = {
        "xT": xT, "w1f": w1f, "c1f": c1f, "w2f": w2f, "c2f": c2f,
        "sel132": sel132.astype(_BF), "sel1": sel1.astype(_BF),
        "sel4to128": sel4to128,
        "selsq": selsq, "sel4to64": sel4to64,
        "espread": espread.astype(_BF), "maskd": maskd.astype(_BF),
        "ident128": np.eye(128, dtype=fl),
    }

    in_maps = []
    for c in range(N_CORES):
        m = dict(shared)
        ks = slice(2 * c, 2 * c + 2)
        # wbT[p=ch, (k, oc, o)] = wbp[2c+k, 128*oc+o, ch]
        m["wbT"] = np.ascontiguousarray(
            wbp[ks].reshape(2, 8, 128, 128).transpose(3, 0, 1, 2)
            .reshape(128, 2048))
        # cb[p, (oc, k)] = cbv[2c+k, 128*oc+p]
        m["cb"] = np.ascontiguousarray(
            cbv[ks].reshape(2, 8, 128).transpose(2, 1, 0).reshape(128, 16))
        # wc[g, 16j+e, 32o+v] = Wc[o, 128c + 16j + g, e, v]  (i_local = 16j+g)
        wcs = Wc[:, 128 * c:128 * (c + 1)]       # [32, 128, 16, 32]
        m["wc"] = np.ascontiguousarray(
            wcs.reshape(32, 8, 16, 16, 32)       # [o, j, g, e, v]
            .transpose(2, 1, 3, 0, 4)            # [g, j, e, o, v]
            .reshape(16, 128, 1024)).astype(_BF)
        in_maps.append(m)
    return in_maps


def host_finish(parts):
    """Sum per-core routing partials s2[b,1024] and apply the final squash."""
    s = np.sum([np.asarray(p, dtype=np.float32) for p in parts], axis=0)
    s = s.reshape(B, 32, 32).astype(np.float64)
    n = np.linalg.norm(s, axis=2, keepdims=True)
    return (s * (n / (1.0 + n * n))).astype(np.float32)


def kernel(**inputs):
    if "nc" not in _CACHE:
        _CACHE["nc"] = _build_bass()
    nc = _CACHE["nc"]
    in_maps = _prepare_inputs(**inputs)
    res = bass_utils.run_bass_kernel_spmd(
        nc, in_maps, core_ids=list(range(N_CORES)))
    return host_finish([r["out"] for r in res.results])



# revision 28
# speedup vs baseline: 1.2873x; 1.2730x over previous
"""Trainium2 Bass kernel for nn_Encoder_55688545960036.

Network: pointnet-style shared MLP (3->64->128, eval-mode BN folded into the
weights on the host, relu), 16 branch matmuls 128->1024 with folded BN and a
maxpool over the 2048 points of each batch element, squash over the branch
axis, capsule transform u[b,o,i,v] = sum_e caps[b,i,e] * Wc[o,i,e,v], 3 rounds
of dynamic routing, squash -> [4,32,32] output.

Distribution over 8 NeuronCores:
  phase A: branch axis k=16 -> 2 branches per core (shared MLP replicated).
  phase B: in-capsule axis i=1024 -> 128 per core (Wc 67MB -> 8.4MB/core).
  collectives: one AllToAll that converts per-core (2 branches, all 1024 i)
  feat into per-core (all 16 branches, 128-i shard), then 3 AllReduces of the
  routing partial sums s[4,32,32] (one per routing iteration).

All heavy matmuls run in float32r (full PE rate, ~1.5e-4 rel err on HW).
The maxpool is fused into PSUM evacuation with tensor_scalar accum max.
The capsule einsum is done as 16 block-diagonal matmuls (8 capsules of the
i-shard per group, caps entries on the block diagonal) with PE column tiling.
Routing keeps (i,b) on partitions and (o,v) in the free dims so the softmax
over o is a free-axis op; sums over i (partitions) are tiny selector matmuls.
"""

import functools

import numpy as np
from contextlib import ExitStack

import concourse.bass as bass
import concourse.tile as tile
from concourse import bacc, mybir
from concourse import bass_utils

# All ACT functions this kernel uses (Relu/Copy/Square/Ln/Exp/Identity) live
# together in the "natural_log_exp_and_others" table set, but the act-table
# placement pass greedily picks the FIRST set containing each func, which
# ping-pongs between Ln-only and Exp-only sets (one 1.3us table load per
# switch). Empty every other set (preserving dict order so walrus's
# act_func_set_id mapping is unchanged) so all activations resolve to the one
# set and a single load suffices.
_KEEP_ACT_SET = "natural_log_exp_and_others"
_orig_gat = bacc.get_activation_tables


@functools.cache
def _patched_gat(arch):
    t = _orig_gat(arch)
    return {k: (v if k == _KEEP_ACT_SET else set()) for k, v in t.items()}


bacc.get_activation_tables = _patched_gat

EPS = 1e-5
N_CORES = 8
B = 4
BN_ = 4 * 2048  # 8192 points
f32 = mybir.dt.float32
f32r = mybir.dt.float32r
AL = mybir.AluOpType
AF = mybir.ActivationFunctionType
AX = mybir.AxisListType
bf16 = mybir.dt.bfloat16
_BF = mybir.dt.np(bf16)

_CACHE = {}

# maxpool evacuation paths: D = DVE-direct-from-psum, A = ACT-copy +
# DVE remax (bf16, 4x), P = Pool(GpSimd)-direct-from-psum. Weighted
# round-robin so DVE/ACT/Pool busy times balance against the PE.
_PATH_W = {"A": 7, "D": 4, "P": 5}


def _make_paths(n=128):
    paths, cnt = [], {k: 0 for k in _PATH_W}
    for i in range(n):
        pick = max(_PATH_W, key=lambda k: _PATH_W[k] * (i + 1) / 16 - cnt[k])
        paths.append(pick)
        cnt[pick] += 1
    return paths


MAXPOOL_PATHS = _make_paths()


def _build_bass(reps=1, debug=False, stage=4, nocoll=False):
    # stage: 1=MLP only, 2=+branch/maxpool, 3=+A2A/caps/u, 4=full (routing)
    # nocoll: replace collectives with local DRAM copies (for TimelineSim)
    # reps > 1 replicates the compute body end-to-end inside one NEFF; used
    # only for wall-clock-difference timing in the dev harness.
    nc = bacc.Bacc("TRN2", target_bir_lowering=False, debug=False,
                   num_devices=N_CORES)

    # ---- DRAM I/O ----
    d_xT = nc.dram_tensor("xT", [3, BN_], f32r, kind="ExternalInput").ap()
    d_w1f = nc.dram_tensor("w1f", [3, 64], f32r, kind="ExternalInput").ap()
    d_c1f = nc.dram_tensor("c1f", [64, 1], f32, kind="ExternalInput").ap()
    d_w2f = nc.dram_tensor("w2f", [64, 128], f32r, kind="ExternalInput").ap()
    d_c2f = nc.dram_tensor("c2f", [128, 1], f32, kind="ExternalInput").ap()
    d_wbT = nc.dram_tensor("wbT", [128, 2048], f32r, kind="ExternalInput").ap()
    d_cb = nc.dram_tensor("cb", [128, 16], f32, kind="ExternalInput").ap()
    d_wc = nc.dram_tensor("wc", [16, 128, 1024], bf16, kind="ExternalInput").ap()
    d_sel132 = nc.dram_tensor("sel132", [128, 4], bf16, kind="ExternalInput").ap()
    d_sel1 = nc.dram_tensor("sel1", [128, 4], bf16, kind="ExternalInput").ap()
    d_sel4to128 = nc.dram_tensor("sel4to128", [4, 128], f32r,
                                 kind="ExternalInput").ap()
    d_selsq = nc.dram_tensor("selsq", [64, 4], f32r, kind="ExternalInput").ap()
    d_sel4to64 = nc.dram_tensor("sel4to64", [4, 64], f32r,
                                kind="ExternalInput").ap()
    d_espread = nc.dram_tensor("espread", [64, 512], bf16,
                               kind="ExternalInput").ap()
    d_maskd = nc.dram_tensor("maskd", [128, 512], bf16,
                             kind="ExternalInput").ap()
    d_ident = nc.dram_tensor("ident128", [128, 128], f32,
                             kind="ExternalInput").ap()
    # "out" holds this core's iteration-2 routing partial s2[b, (o,v)];
    # the host sums across cores and applies the final squash.
    d_out = nc.dram_tensor("out", [B, 1024], f32, kind="ExternalOutput").ap()
    d_dbg = {}
    if debug:
        for nm, shp in [("h2T", [128, BN_]), ("feat", [128, 64]),
                        ("FT", [64, 128]), ("capsT", [64, 128]),
                        ("lhsT", [128, 512]), ("u0", [128, 1024]),
                        ("u1", [128, 1024]), ("u2", [128, 1024]),
                        ("u3", [128, 1024]), ("blog0", [128, 128]),
                        ("sg0", [4, 1024]), ("a0", [4, 1024]),
                        ("c1it", [128, 128]), ("sg1", [4, 1024])]:
            d_dbg[nm] = nc.dram_tensor("dbg_" + nm, shp, f32,
                                       kind="ExternalOutput").ap()

    # collective bounce buffers (internal DRAM); A2A operates on first-dim
    # blocks: in[j] goes to rank j, out[r] came from rank r.
    d_a2a_in = [nc.dram_tensor(f"a2a_in_r{r}", [8, 2, B, 128], f32)
                for r in range(reps)]
    d_a2a_out = [nc.dram_tensor(f"a2a_out_r{r}", [8, 2, B, 128], f32)
                 for r in range(reps)]
    d_s_in = [[nc.dram_tensor(f"s_in{t}_r{r}", [B, 1024], f32)
               for t in range(3)] for r in range(reps)]
    d_s_out = [[nc.dram_tensor(f"s_out{t}_r{r}", [B, 1024], f32,
                               addr_space="Shared") for t in range(3)]
               for r in range(reps)]

    rg = [list(range(N_CORES))]

    with tile.TileContext(nc) as tc, ExitStack() as ctx:
        const = ctx.enter_context(tc.tile_pool(name="const", bufs=1))
        big = ctx.enter_context(tc.tile_pool(name="big", bufs=1))
        work = ctx.enter_context(tc.tile_pool(name="work", bufs=2))
        small = ctx.enter_context(tc.tile_pool(name="small", bufs=1))

        # ---- load constants / weights ----
        def load_const(name, dram, shape, dt, eng=None):
            t = const.tile(shape, dt, name=name)
            (eng or nc.sync).dma_start(out=t, in_=dram)
            return t

        xT = load_const("xT_sb", d_xT, [3, BN_], f32r)
        w1f = load_const("w1f_sb", d_w1f, [3, 64], f32r)
        c1f = load_const("c1f_sb", d_c1f, [64, 1], f32)
        w2f = load_const("w2f_sb", d_w2f, [64, 128], f32r)
        c2f = load_const("c2f_sb", d_c2f, [128, 1], f32)
        wbT = load_const("wbT_sb", d_wbT, [128, 2048], f32r)
        cb = load_const("cb_sb", d_cb, [128, 16], f32)
        sel132 = load_const("sel132_sb", d_sel132, [128, 4], bf16)
        sel1 = load_const("sel1_sb", d_sel1, [128, 4], bf16)
        sel4to128 = load_const("sel4to128_sb", d_sel4to128, [4, 128], f32r)
        selsq = load_const("selsq_sb", d_selsq, [64, 4], f32r)
        sel4to64 = load_const("sel4to64_sb", d_sel4to64, [4, 64], f32r)
        espread = load_const("espread_sb", d_espread, [64, 512], bf16)
        maskd = load_const("maskd_sb", d_maskd, [128, 512], bf16)

        ident = load_const("ident_sb", d_ident, [128, 128], f32)
        wc_all = const.tile([128, 16384], bf16, name="wc_all_sb")
        nc.sync.dma_start(
            out=wc_all.rearrange("p (g f) -> p g f", g=16),
            in_=d_wc.rearrange("g p f -> p g f"))
        wc_sb = [wc_all[:, 1024 * g:1024 * (g + 1)] for g in range(16)]

        def _body(rep):
            h2T = big.tile([128, BN_], f32r)  # [channel, point]

            # ---- phase A: shared MLP (own psum scope), then branch
            # matmuls + fused maxpool with all 8 psum banks as single-bank
            # tiles so the three evacuation engines never starve the PE.
            feat_sb = big.tile([128, 64], f32)  # [o_in_chunk, (k, oc, b)]
            feat_pt = big.tile([128, 256], f32)  # per-quarter partials
            with tc.tile_pool(name="ps_mlp", bufs=4, space="PSUM") as ps_mlp:
                for j in range(16):
                    sl = bass.ts(j, 512)
                    p1 = ps_mlp.tile([64, 512], f32, tag="p1")
                    nc.tensor.matmul(p1, w1f, xT[:, sl], start=True,
                                     stop=True)
                    h1c = work.tile([64, 512], f32r, tag="h1c", bufs=4)
                    if j % 2 == 0:
                        nc.scalar.activation(out=h1c, in_=p1, func=AF.Relu,
                                             bias=c1f, scale=1.0)
                    else:
                        nc.vector.tensor_scalar(out=h1c, in0=p1, scalar1=c1f,
                                                scalar2=0.0, op0=AL.add,
                                                op1=AL.max)
                    p2 = ps_mlp.tile([128, 512], f32, tag="p2")
                    nc.tensor.matmul(p2, w2f, h1c, start=True, stop=True)
                    if j % 2 == 0:
                        nc.vector.tensor_scalar(out=h2T[:, sl], in0=p2,
                                                scalar1=c2f, scalar2=0.0,
                                                op0=AL.add, op1=AL.max)
                    else:
                        nc.scalar.activation(out=h2T[:, sl], in_=p2,
                                             func=AF.Relu, bias=c2f,
                                             scale=1.0)
            if stage > 1:
                with tc.tile_pool(name="ps_y", bufs=8, space="PSUM") as ps_y:
                    idx = 0
                    for k in range(2):
                        for oc in range(8):
                            lw = wbT[:, bass.ts(k * 8 + oc, 128)]
                            for b in range(B):
                                for q in range(4):
                                    py = ps_y.tile([128, 512], f32, tag="py")
                                    nc.tensor.matmul(
                                        py, lw,
                                        h2T[:, bass.ts(4 * b + q, 512)],
                                        start=True, stop=True)
                                    s3 = (((oc * 2 + k) * 4 + b) * 4 + q)
                                    acc = feat_pt[:, s3:s3 + 1]
                                    path = MAXPOOL_PATHS[
                                        idx % len(MAXPOOL_PATHS)]
                                    idx += 1
                                    if path == "A":
                                        # ACT evac to bf16; DVE remax (4x)
                                        pair = work.tile([128, 512], bf16,
                                                         tag="pair", bufs=10)
                                        nc.scalar.copy(pair, py)
                                        nc.vector.tensor_scalar(
                                            out=pair, in0=pair,
                                            scalar1=-3.0e38, scalar2=None,
                                            op0=AL.max, op1=AL.max,
                                            accum_out=acc)
                                    else:
                                        # DVE or Pool direct from psum; ALU
                                        # out to SBUF scratch (psum read-only)
                                        eng = (nc.vector if path == "D"
                                               else nc.gpsimd)
                                        junk = work.tile(
                                            [128, 512], f32,
                                            tag="junk" + path, bufs=2)
                                        eng.tensor_scalar(
                                            out=junk, in0=py, scalar1=-3.0e38,
                                            scalar2=None, op0=AL.max,
                                            op1=AL.max, accum_out=acc)

            if stage <= 1:
                nc.sync.dma_start(d_out, h2T[0:B, 0:1024].bitcast(f32))
                return
            # combine the four quarter-maxes per block
            nc.vector.tensor_reduce(
                feat_sb, feat_pt.rearrange("p (s q) -> p s q", q=4),
                axis=AX.X, op=AL.max)

            # feat += cb (cb[p, (oc, k)] broadcast over b)
            feat_v = feat_sb.rearrange("p (oc k b) -> p oc k b", oc=8, k=2)
            cb_bc = cb.rearrange("p (oc k) -> p oc k", oc=8).unsqueeze(3) \
                      .broadcast_to((128, 8, 2, 4))
            nc.vector.tensor_add(feat_v, feat_v, cb_bc)

            if debug and rep == 0:
                nc.sync.dma_start(d_dbg["feat"], feat_sb)
            if stage <= 2:
                nc.sync.dma_start(d_out[:, 0:32], feat_sb[0:B, 0:32])
                return
            # transpose feat on the PE so the a2a_in DMA is one contiguous
            # 32KB copy (featT flat layout == a2a_in flat layout).
            with tc.tile_pool(name="ps_ft", bufs=1, space="PSUM") as ps_ft:
                p_ftr = ps_ft.tile([64, 128], f32, tag="pft")
                nc.tensor.transpose(p_ftr, feat_sb, ident)
                featT = work.tile([64, 128], f32, tag="featT")
                nc.vector.tensor_copy(featT, p_ftr)
            nc.sync.dma_start(d_a2a_in[rep].ap(), featT)

            # ---- AllToAll: out viewed [16(e), B, 128(i_local)] ----
            if nocoll:
                nc.sync.dma_start(d_a2a_out[rep].ap(), d_a2a_in[rep].ap())
            else:
                nc.gpsimd.collective_compute(
                    "AllToAll", AL.bypass, ins=[d_a2a_in[rep].ap().opt()],
                    outs=[d_a2a_out[rep].ap().opt()], replica_groups=rg)

            # ---- phase B ----
            with tc.tile_pool(name="ps_b", bufs=2, space="PSUM") as ps_b, \
                 tc.tile_pool(name="ps_tiny", bufs=1, space="PSUM") as ps_tiny, \
                 tc.tile_pool(name="ps_s", bufs=1, space="PSUM") as ps_s:

                # caps: squash over branch axis e. FT[q = 4e+b, i_local]:
                # a2a_out flat row (r,k,b) = 4*(2r+k)+b = 4e+b, so the whole
                # tile is one contiguous 32KB DMA.
                FT = big.tile([64, 128], f32)
                nc.sync.dma_start(
                    FT, d_a2a_out[rep].ap().rearrange("r k b l -> (r k b) l"))

                FT2 = work.tile([64, 128], f32r, tag="ft2")
                nc.vector.tensor_mul(FT2, FT, FT)
                p_n2 = ps_tiny.tile([4, 128], f32, tag="pp")
                nc.tensor.matmul(p_n2, selsq, FT2,
                                 start=True, stop=True)
                # factor = sqrt(n2)/(1+n2) = exp(0.5*ln(n2) - ln(1+n2));
                # Ln/Exp/Relu/Copy/Square share one act table set, so no
                # act-table reloads anywhere in the kernel.
                l1 = small.tile([4, 128], f32, tag="l1")
                nc.scalar.activation(out=l1, in_=p_n2, func=AF.Ln, bias=0.0,
                                     scale=1.0)
                l2 = small.tile([4, 128], f32, tag="l2")
                nc.scalar.activation(out=l2, in_=p_n2, func=AF.Ln, bias=1.0,
                                     scale=1.0)
                ld = small.tile([4, 128], f32, tag="ld")
                nc.vector.scalar_tensor_tensor(
                    out=ld, in0=l1, scalar=0.5, in1=l2,
                    op0=AL.mult, op1=AL.subtract)
                fct = small.tile([4, 128], f32r, tag="fct")
                nc.scalar.activation(out=fct, in_=ld, func=AF.Exp, bias=0.0,
                                     scale=1.0)
                p_fbc = ps_tiny.tile([64, 128], f32, tag="pp")
                nc.tensor.matmul(p_fbc, sel4to64, fct,
                                 start=True, stop=True)
                capsT = work.tile([64, 128], bf16, tag="capsT")
                nc.vector.tensor_mul(capsT, FT, p_fbc)
                if debug and rep == 0:
                    nc.sync.dma_start(d_dbg["FT"], FT)
                    nc.gpsimd.dma_start(out=d_dbg["capsT"], in_=capsT)

                # block-diagonal lhsT[16j+e, 32g+4j'+b] = caps[b, 16j'+g, e]
                # * delta(j==j') via PE spread + masked mul (no DMAs):
                # OUT4[16j+e, 128b + i] = caps[b, i, e] (j-replicated).
                p_sp = ps_tiny.tile([128, 512], f32, tag="psp")
                for b in range(4):
                    nc.tensor.matmul(p_sp[:, bass.ts(b, 128)],
                                     espread[:, bass.ts(b, 128)], capsT,
                                     start=True, stop=True)
                lhsT = big.tile([128, 512], bf16)
                nc.vector.tensor_mul(
                    lhsT.rearrange("p (g j b) -> p g j b", g=16, j=8, b=4),
                    p_sp.rearrange("p (b j g) -> p g j b", b=4, j=8, g=16),
                    maskd.rearrange("p (g j b) -> p g j b", g=16, j=8, b=4))

                if debug and rep == 0:
                    nc.gpsimd.dma_start(out=d_dbg["lhsT"], in_=lhsT)
                # u matmuls: 16 groups of 8 capsules; 4 groups col-tiled
                # per psum tile. u_all[p = 32q + 4j + b, (t, o, v)] in bf16,
                # i_local = 16j + (4t+q)
                u_all = big.tile([128, 4096], bf16)
                u_evac = [nc.vector, nc.scalar, nc.gpsimd, nc.scalar]
                # ps0 accumulates the iteration-0 partial s0 = sum_i u/32;
                # its selector matmuls interleave with u production so the
                # first AllReduce can launch as soon as the last u tile
                # lands (accumulation groups on separate psum banks).
                ps0 = ps_s.tile([4, 1024], f32, tag="ps")
                for t in range(4):
                    pu = ps_b.tile([128, 1024], f32, tag="pu")
                    for q in range(4):
                        g = 4 * t + q
                        for h in range(2):
                            nc.tensor.matmul(
                                pu[32 * q:32 * q + 32, bass.ts(h, 512)],
                                lhsT[:, bass.ts(g, 32)],
                                wc_sb[g][:, bass.ts(h, 512)],
                                start=True, stop=True,
                                tile_position=(0, 32 * q))
                    if t == 1 or t == 3:
                        nc.scalar.copy(u_all[:, bass.ts(t, 1024)], pu)
                    else:
                        u_evac[t].tensor_copy(u_all[:, bass.ts(t, 1024)], pu)
                    for h in range(2):
                        nc.tensor.matmul(
                            ps0[:, bass.ts(h, 512)], sel132,
                            u_all[:, 1024 * t + 512 * h:
                                  1024 * t + 512 * (h + 1)],
                            start=(t == 0), stop=(t == 3),
                            skip_group_check=True)
                    if debug and rep == 0:
                        nc.gpsimd.dma_start(out=d_dbg[f"u{t}"],
                                            in_=u_all[:, bass.ts(t, 1024)])

                # ---- routing ----
                b_log = big.tile([128, 128], f32)  # [(q,j,b), (t,o)]
                uv = u_all.rearrange("p (t o v) -> p t o v", t=4, v=32)

                def s_round(c_sb, sel, pst):
                    # pst[4, 1024] = sum_t sel.T @ (u[:, t] * c_bc) — the
                    # weighted mul of tile t is interleaved with its two
                    # accumulating matmuls so PE overlaps DVE.
                    for t in range(4):
                        if c_sb is None:
                            wt_t = u_all[:, bass.ts(t, 1024)]
                        else:
                            wtt = work.tile([128, 1024], bf16, tag="wt",
                                            bufs=4)
                            nc.vector.tensor_mul(
                                wtt.rearrange("p (o v) -> p o v", v=32),
                                uv[:, t, :, :],
                                c_sb[:, bass.ts(t, 32)].unsqueeze(2)
                                    .broadcast_to((128, 32, 32)))
                            wt_t = wtt
                        for h in range(2):
                            nc.tensor.matmul(
                                pst[:, bass.ts(h, 512)], sel,
                                wt_t[:, bass.ts(h, 512)],
                                start=(t == 0), stop=(t == 3))

                def allreduce_s(pst, it):
                    s_loc = small.tile([4, 1024], f32, tag="s_loc")
                    nc.scalar.copy(s_loc, pst)
                    nc.sync.dma_start(d_s_in[rep][it].ap(), s_loc)
                    if nocoll:
                        nc.sync.dma_start(d_s_out[rep][it].ap(),
                                          d_s_in[rep][it].ap())
                    else:
                        nc.gpsimd.collective_compute(
                            "AllReduce", AL.add,
                            ins=[d_s_in[rep][it].ap().opt()],
                            outs=[d_s_out[rep][it].ap().opt()],
                            replica_groups=rg)
                    s_glob = small.tile([4, 1024], f32, tag=f"s_glob{it}")
                    nc.scalar.dma_start(out=s_glob, in_=d_s_out[rep][it].ap())
                    return s_glob

                def broadcast_s(s_glob):
                    # s_bc[p,(o,v)] = s_glob[b(p),(o,v)]; runs on PE in
                    # parallel with the squash-factor chain.
                    p_s = ps_b.tile([128, 1024], f32, tag="pu")
                    sgr = s_glob.bitcast(f32r)
                    for h in range(2):
                        nc.tensor.matmul(p_s[:, bass.ts(h, 512)], sel4to128,
                                         sgr[:, bass.ts(h, 512)],
                                         start=True, stop=True)
                    return p_s

                def squash_factor(s_glob):
                    # f[b,o] = |s|/(1+|s|^2) = exp(0.5*ln(n2) - ln(1+n2))
                    s2 = small.tile([4, 1024], f32, tag="sq_s2")
                    nc.scalar.square(s2, s_glob)
                    sn2 = small.tile([4, 32], f32, tag="sq_n2")
                    nc.vector.reduce_sum(
                        sn2, s2.rearrange("p (o v) -> p o v", v=32), axis=AX.X)
                    sl1 = small.tile([4, 32], f32, tag="sq_l1")
                    nc.scalar.activation(out=sl1, in_=sn2, func=AF.Ln,
                                         bias=0.0, scale=1.0)
                    sl2 = small.tile([4, 32], f32, tag="sq_l2")
                    nc.scalar.activation(out=sl2, in_=sn2, func=AF.Ln,
                                         bias=1.0, scale=1.0)
                    sld = small.tile([4, 32], f32, tag="sq_ld")
                    nc.vector.scalar_tensor_tensor(
                        out=sld, in0=sl1, scalar=0.5, in1=sl2,
                        op0=AL.mult, op1=AL.subtract)
                    sf = small.tile([4, 32], f32r, tag="sq_f")
                    nc.scalar.activation(out=sf, in_=sld, func=AF.Exp,
                                         bias=0.0, scale=1.0)
                    return sf

                def agree_update(s_bc, sf, first):
                    # abc[p,(o,v)] = squash(s)[b(p),(o,v)] broadcast:
                    # s_bc (done during the norm chain) times f_bc, fused.
                    p_f = ps_tiny.tile([128, 32], f32, tag="psp")
                    nc.tensor.matmul(p_f, sel4to128, sf, start=True, stop=True)
                    abc_sb = work.tile([128, 1024], bf16, tag="abc")
                    nc.vector.tensor_mul(
                        abc_sb.rearrange("p (o v) -> p o v", v=32),
                        s_bc.rearrange("p (o v) -> p o v", v=32),
                        p_f.unsqueeze(2).broadcast_to((128, 32, 32)))
                    abc_bc = abc_sb.rearrange("p (o v) -> p o v", v=32)
                    for t in range(4):
                        tmp = work.tile([128, 1024], bf16, tag="tmp")
                        tv = tmp.rearrange("p (o v) -> p o v", v=32)
                        nc.vector.tensor_mul(
                            tv, uv[:, t, :, :], abc_bc)
                        if first:
                            nc.vector.reduce_sum(
                                b_log[:, bass.ts(t, 32)], tv, axis=AX.X)
                        else:
                            agr = work.tile([128, 32], f32, tag="agr")
                            nc.vector.reduce_sum(agr, tv, axis=AX.X)
                            nc.vector.tensor_add(b_log[:, bass.ts(t, 32)],
                                                 b_log[:, bass.ts(t, 32)],
                                                 agr)

                def softmax_c():
                    cexp = work.tile([128, 128], f32, tag="cexp")
                    nc.scalar.activation(out=cexp, in_=b_log, func=AF.Exp,
                                         bias=0.0, scale=1.0)
                    sums = small.tile([128, 4], f32, tag="csum")
                    nc.vector.reduce_sum(
                        sums, cexp.rearrange("p (t o) -> p t o", o=32),
                        axis=AX.X)
                    crec = small.tile([128, 4], f32, tag="crec")
                    nc.vector.reciprocal(crec, sums)
                    c_sb = work.tile([128, 128], bf16, tag="c_sb")
                    nc.vector.tensor_mul(
                        c_sb.rearrange("p (t o) -> p t o", o=32),
                        cexp.rearrange("p (t o) -> p t o", o=32),
                        crec.unsqueeze(2).broadcast_to((128, 4, 32)))
                    return c_sb

                # iteration 0: ps0 was accumulated inside the u loop
                sg0 = allreduce_s(ps0, 0)
                if debug and rep == 0:
                    nc.sync.dma_start(d_dbg["sg0"], sg0)
                sbc0 = broadcast_s(sg0)
                agree_update(sbc0, squash_factor(sg0), first=True)
                if debug and rep == 0:
                    nc.sync.dma_start(d_dbg["blog0"], b_log)

                # iteration 1
                c1it = softmax_c()
                if debug and rep == 0:
                    nc.sync.dma_start(d_dbg["c1it"], c1it)
                ps1 = ps_s.tile([4, 1024], f32, tag="ps")
                s_round(c1it, sel1, ps1)
                sg1 = allreduce_s(ps1, 1)
                if debug and rep == 0:
                    nc.sync.dma_start(d_dbg["sg1"], sg1)
                sbc1 = broadcast_s(sg1)
                agree_update(sbc1, squash_factor(sg1), first=False)

                # iteration 2 (final): local partial s only; the host
                # all-reduces across cores and applies the final squash.
                ps2 = ps_s.tile([4, 1024], f32, tag="ps")
                s_round(softmax_c(), sel1, ps2)
                s_out2 = small.tile([4, 1024], f32, tag="s_out2")
                nc.scalar.copy(s_out2, ps2)
                nc.sync.dma_start(d_out, s_out2)

        for _rep in range(reps):
            _body(_rep)


    nc.compile()
    return nc


def _prepare_inputs(x, w1, g1, b1, m1, v1, w2, g2, b2, m2, v2,
                    wb, gb, bb, mb, vb, Wc):
    """Host-side: fold BN into weights, transpose/shard for the device."""
    fl = np.float32
    x = np.asarray(x, fl); w1 = np.asarray(w1, fl); w2 = np.asarray(w2, fl)
    wb = np.asarray(wb, fl); Wc = np.asarray(Wc, fl)
    g1, b1, m1, v1 = (np.asarray(a, fl) for a in (g1, b1, m1, v1))
    g2, b2, m2, v2 = (np.asarray(a, fl) for a in (g2, b2, m2, v2))
    gb, bb, mb, vb = (np.asarray(a, fl) for a in (gb, bb, mb, vb))

    s1 = g1 / np.sqrt(v1 + EPS)
    c1 = b1 - m1 * s1
    w1f = (w1 * s1[:, None]).T.copy()            # [3, 64]
    c1f = np.ascontiguousarray(c1[:, None])

    s2 = g2 / np.sqrt(v2 + EPS)
    c2 = b2 - m2 * s2
    w2f = (w2 * s2[:, None]).T.copy()            # [64, 128]
    c2f = np.ascontiguousarray(c2[:, None])

    sb = gb / np.sqrt(vb + EPS)                  # [16, 1024]
    wbp = wb * sb[:, :, None]                    # [16, 1024, 128]
    cbv = bb - mb * sb                           # [16, 1024]

    xT = np.ascontiguousarray(x.reshape(BN_, 3).T)  # [3, 8192]

    p = np.arange(128)
    sel1 = ((p[:, None] % 4) == np.arange(4)[None, :]).astype(fl)
    sel132 = sel1 / 32.0
    sel4to128 = np.ascontiguousarray(sel1.T)
    # FT rows are q = 4e + b, so the per-b selectors key on q % 4
    q64 = np.arange(64)
    selsq = ((q64[:, None] % 4) == np.arange(4)[None, :]).astype(fl)
    sel4to64 = np.ascontiguousarray(selsq.T)
    # espread[4e+b, 128b2 + 16j + e2] = (b==b2)&(e==e2) for all j:
    # spreads capsT rows (4e+b) onto partitions (16j+e) per b-block.
    espread = np.zeros((64, 512), fl)
    for e in range(16):
        for b in range(4):
            for j in range(8):
                espread[4 * e + b, 128 * b + 16 * j + e] = 1.0
    # maskd[16j+e, 32g+4j2+b] = (j == j2)
    maskd = np.zeros((128, 512), fl)
    for j in range(8):
        for e in range(16):
            maskd[16 * j + e, np.arange(16) * 32 + 4 * j + np.arange(4)[:, None]] = 1.0

    shared = {
        "xT": xT, "w1f": w1f, "c1f": c1f, "w2f": w2f, "c2f": c2f,
        "sel132": sel132.astype(_BF), "sel1": sel1.astype(_BF),
        "sel4to128": sel4to128,
        "selsq": selsq, "sel4to64": sel4to64,
        "espread": espread.astype(_BF), "maskd": maskd.astype(_BF),
        "ident128": np.eye(128, dtype=fl),
    }

    in_maps = []
    for c in range(N_CORES):
        m = dict(shared)
        ks = slice(2 * c, 2 * c + 2)
        # wbT[p=ch, (k, oc, o)] = wbp[2c+k, 128*oc+o, ch]
        m["wbT"] = np.ascontiguousarray(
            wbp[ks].reshape(2, 8, 128, 128).transpose(3, 0, 1, 2)
            .reshape(128, 2048))
        # cb[p, (oc, k)] = cbv[2c+k, 128*oc+p]
        m["cb"] = np.ascontiguousarray(
            cbv[ks].reshape(2, 8, 128).transpose(2, 1, 0).reshape(128, 16))
        # wc[g, 16j+e, 32o+v] = Wc[o, 128c + 16j + g, e, v]  (i_local = 16j+g)
        wcs = Wc[:, 128 * c:128 * (c + 1)]       # [32, 128, 16, 32]
        m["wc"] = np.ascontiguousarray(
            wcs.reshape(32, 8, 16, 16, 32)       # [o, j, g, e, v]
            .transpose(2, 1, 3, 0, 4)            # [g, j, e, o, v]
            .reshape(16, 128, 1024)).astype(_BF)
        in_maps.append(m)
    return in_maps


def host_finish(parts):
    """Sum per-core routing partials s2[b,1024] and apply the final squash."""
    s = np.sum([np.asarray(p, dtype=np.float32) for p in parts], axis=0)
    s = s.reshape(B, 32, 32).astype(np.float64)
    n = np.linalg.norm(s, axis=2, keepdims=True)
    return (s * (n / (1.0 + n * n))).astype(np.float32)


def kernel(**inputs):
    if "nc" not in _CACHE:
        _CACHE["nc"] = _build_bass()
    nc = _CACHE["nc"]
    in_maps = _prepare_inputs(**inputs)
    res = bass_utils.run_bass_kernel_spmd(
        nc, in_maps, core_ids=list(range(N_CORES)))
    return host_finish([r["out"] for r in res.results])

